# revision 1
# baseline (speedup 1.0000x reference)
"""Causal multi-head attention (RoPE) forward for Trainium2, 8 NeuronCores.

Problem: B=2, T=2048, C=1024, H=16, D=64.  out = proj(softmax(rope(q) rope(k)^T / 8, causal) @ v)

Sharding: 8 cores = 2 batches x 4 head-groups (4 heads each).
 - qkv projection column-sharded per head group, proj row-sharded; host sums
   the 4 per-group partial projections per batch.
 - All device matmuls run transposed layouts so no on-device transposes are
   needed:
     q^T,k^T   [d, t]  <- w^T-block as lhsT, x^T as rhs
     v         [t, d]  <- x^T-block as lhsT, w^T v-cols as rhs
     S^T       [j, i]  <- k^T-block as lhsT, q^T as rhs       (j keys, i queries)
     P^T       = exp(S^T/8)  (no max subtraction: |scores| <= ~3 by construction)
     y_aug^T   [65, i] <- v_aug (ones column -> softmax denominator) as lhsT,
                          P^T as rhs, PSUM-accumulated over j blocks
     out       [t, o]  <- ynorm^T-block as lhsT, wproj^T as rhs
 - RoPE: q_rope = q*cos + R(q*sinP), with sinP a half-swapped sin table and
   R the block-diagonal rotate-half matrix applied by one 128x128 PE matmul
   per tile -- 3 full-width DVE passes total, no partition shifts.
 - Causality: key-blocks above the diagonal are skipped entirely; diagonal
   blocks are masked after the exp; the uppermost diagonal span computes
   only the surviving query sub-ranges (partial QK/exp/AV).
 - Single fused chunk pipeline: qkv(tc) -> rope(tc) -> attention(ic=tc) with
   the output projection deferred to fill tail PE gaps.
"""

import numpy as np
import ml_dtypes

_CACHE = {}

B, T, C = 2, 2048, 1024
HLOC, D = 4, 64            # heads per core, head dim
GC = HLOC * D              # 256 channels per group
P = 128
NTT = T // P               # 16 key/row tiles
NIC = T // 512             # 4 query chunks of 512
THETA = 10000.0
N_CORES = 8


def _rope_tables():
    freqs = 1.0 / THETA ** (np.arange(0, D, 2, dtype=np.float32) / D)
    t = np.arange(T, dtype=np.float32)
    f = np.outer(t, freqs)                          # [T, 32]
    emb = np.concatenate([f, f], axis=-1)           # [T, 64]
    cosT = np.cos(emb).T.astype(np.float32)         # [64, T]
    sinT = np.sin(emb).T.astype(np.float32)
    # tile to 128 partitions (2 heads per partition block)
    return (np.concatenate([cosT, cosT], 0), np.concatenate([sinT, sinT], 0))


def _masks():
    # mask_m[j, i'] = 1 if i' >= j + 128*m   (keep key j_abs <= query i_abs)
    j = np.arange(P)[:, None]
    i = np.arange(512)[None, :]
    return np.stack([(i >= j + P * m) for m in range(4)]).astype(np.float32)  # [4,128,512]


def _build_program():
    import concourse.bass as bass
    import concourse.mybir as mybir
    import concourse.tile as tile

    dt = mybir.dt
    fp32 = dt.float32
    bf16 = dt.bfloat16
    EXP = mybir.ActivationFunctionType.Exp
    MUL = mybir.AluOpType.mult

    nc = bass.Bass("TRN2", target_bir_lowering=False, debug=False,
                   enable_asserts=True, num_devices=N_CORES)

    xT = nc.dram_tensor("xT", [C, T], bf16, kind="ExternalInput").ap()
    wT = nc.dram_tensor("wT", [C, 3 * GC], bf16, kind="ExternalInput").ap()        # q,k,v col-blocks
    rmat_d = nc.dram_tensor("rmat", [P, P], bf16, kind="ExternalInput").ap()
    wpT = nc.dram_tensor("wpT", [GC, C], bf16, kind="ExternalInput").ap()
    cosT_d = nc.dram_tensor("cosT", [P, T], bf16, kind="ExternalInput").ap()
    sinT_d = nc.dram_tensor("sinT", [P, T], bf16, kind="ExternalInput").ap()
    masks_d = nc.dram_tensor("masks", [4, P, 512], bf16, kind="ExternalInput").ap()
    out_d = nc.dram_tensor("out", [T, C], fp32, kind="ExternalOutput").ap()

    CO = C // P  # 8 contraction blocks

    with tile.TileContext(nc) as tc:
        with (
            tc.tile_pool(name="persist", bufs=1) as persist,
            tc.tile_pool(name="work", bufs=8) as work,
            tc.tile_pool(name="pt", bufs=20) as ptpool,
            tc.tile_pool(name="outp", bufs=8) as outpool,
            tc.tile_pool(name="qkvp", bufs=2, space="PSUM") as qkvp,
            tc.tile_pool(name="sspan", bufs=2, space="PSUM") as sspan,
            tc.tile_pool(name="yav", bufs=2, space="PSUM") as yav,
        ):
            # ---- persistent SBUF loads -------------------------------------
            # load order = first-use order: w(q,k) cols, x chunk 0, w(Rq,Rk,v),
            # rope tables, remaining x chunks, proj weight, masks
            rmat_sb = persist.tile([P, P], bf16, tag="rmat")
            nc.sync.dma_start(rmat_sb[:], rmat_d[:])
            warm = qkvp.tile([P, P], fp32, tag="qkvp", name="warmup")
            for i in range(40):
                nc.tensor.matmul(warm[:], rmat_sb[:], rmat_sb[:],
                                 start=True, stop=True, skip_group_check=True)
            wqk, wrv = [], []
            xT_sb = [[None] * NIC for _ in range(CO)]
            def load_xchunk(tcix):
                eng = nc.gpsimd
                for co in range(CO):
                    t = persist.tile([P, 512], bf16, tag=f"x{co}_{tcix}", name=f"x{co}_{tcix}")
                    eng.dma_start(t[:], xT[P * co:P * (co + 1), 512 * tcix:512 * (tcix + 1)])
                    xT_sb[co][tcix] = t
            for co in range(CO):          # interleave so MM co-block k can start early
                t = persist.tile([P, 512], bf16, tag=f"wqk{co}", name=f"wqk{co}")
                nc.sync.dma_start(t[:], wT[P * co:P * (co + 1), 0:512])
                wqk.append(t)
                t = persist.tile([P, 512], bf16, tag=f"x{co}_0", name=f"x{co}_0")
                nc.gpsimd.dma_start(t[:], xT[P * co:P * (co + 1), 0:512])
                xT_sb[co][0] = t
            for co in range(CO):
                t = persist.tile([P, 256], bf16, tag=f"wrv{co}", name=f"wrv{co}")
                nc.sync.dma_start(t[:], wT[P * co:P * (co + 1), 512:768])
                wrv.append(t)
            cos_sb = persist.tile([P, T], bf16, tag="cos")
            nc.sync.dma_start(cos_sb[:], cosT_d[:])
            sin_sb = persist.tile([P, T], bf16, tag="sin")
            nc.sync.dma_start(sin_sb[:], sinT_d[:])
            mask_sb = persist.tile([P, 4, 512], bf16, tag="masks")
            nc.sync.dma_start(mask_sb[:], masks_d.rearrange("m p i -> p m i"))
            for tcix in range(1, NIC):
                load_xchunk(tcix)
            wpT_sb = persist.tile([P, 2, C], bf16, tag="wpT")
            nc.sync.dma_start(wpT_sb[:], wpT.rearrange("(cb p) o -> p cb o", p=P))
            ones_sb = persist.tile([1, D], bf16, tag="ones")
            nc.vector.memset(ones_sb[:], 1.0)

            # rope outputs: q^T,k^T per 2-head block [128, T] bf16
            qk_rope = [persist.tile([P, T], bf16, tag=f"qkrope{i}", name=f"qkrope{i}") for i in range(4)]
            # v with ones column per head: [128part=t, 16 ttiles, 4*65]
            v_aug = persist.tile([P, NTT, HLOC * (D + 1)], bf16, tag="vaug")
            nc.vector.memset(v_aug[:], 1.0)
            # normalized y^T blocks (2 heads each) [128, T] bf16
            ynorm = [persist.tile([P, T], bf16, tag=f"ynorm{i}", name=f"ynorm{i}") for i in range(2)]

            def emit_proj(ic, pool, tag):
                for tt in range(4 * ic, 4 * ic + 4):
                    for oc in range(2):
                        ps = pool.tile([P, 512], fp32, tag=tag, name=f"pso_{tt}_{oc}")
                        for cb in range(2):
                            nc.tensor.matmul(
                                ps[:], ynorm[cb][:, 128 * tt:128 * (tt + 1)],
                                wpT_sb[:, cb, 512 * oc:512 * (oc + 1)],
                                start=(cb == 0), stop=(cb == 1))
                        ob = outpool.tile([P, 512], fp32, tag="ob")
                        nc.any.tensor_copy(out=ob[:], in_=ps[:])
                        nc.sync.dma_start(out_d[128 * tt:128 * (tt + 1), 512 * oc:512 * (oc + 1)], ob[:])

            # ---- fused pipeline: per 512-token chunk tc:
            #   qkv(tc) -> rope(tc) -> attention(ic=tc, keys<=tc) -> proj(tc)
            # (causality means chunk tc's attention needs only k/v chunks <= tc,
            #  so the ACT-bound softmax overlaps the PE-bound qkv of later chunks)
            for tcix in range(NIC):
                # q/k projections (ft 0,1 q; 2,3 k) + rotated partners, + rope
                for ft in range(4):
                    ps = qkvp.tile([P, 512], fp32, tag="qkvp", name=f"psq_{ft}_{tcix}")
                    for co in range(CO):
                        nc.tensor.matmul(
                            ps[:], wqk[co][:, 128 * ft:128 * (ft + 1)],
                            xT_sb[co][tcix][:], start=(co == 0), stop=(co == CO - 1))
                    t1 = work.tile([P, 512], bf16, tag="t1")
                    nc.vector.tensor_tensor(t1[:], ps[:], cos_sb[:, 512 * tcix:512 * (tcix + 1)], MUL)
                    u = work.tile([P, 512], bf16, tag="u")
                    nc.vector.tensor_tensor(u[:], ps[:], sin_sb[:, 512 * tcix:512 * (tcix + 1)], MUL)
                    psr = qkvp.tile([P, 512], fp32, tag="qkvp", name=f"psr_{ft}_{tcix}")
                    nc.tensor.matmul(psr[:], rmat_sb[:], u[:], start=True, stop=True)
                    nc.vector.tensor_add(qk_rope[ft][:, 512 * tcix:512 * (tcix + 1)], psr[:], t1[:])
                # v for this chunk's 4 key tiles
                for tt in range(4 * tcix, 4 * tcix + 4):
                    ps = qkvp.tile([P, 512], fp32, tag="qkvp", name=f"psv_{tt}")
                    for co in range(CO):
                        nc.tensor.matmul(
                            ps[:, :GC], xT_sb[co][tt // 4][:, 128 * (tt % 4):128 * (tt % 4 + 1)],
                            wrv[co][:, 0:256], start=(co == 0), stop=(co == CO - 1))
                    nc.any.tensor_copy(
                        out=v_aug[:, tt].rearrange("p (h e) -> p h e", e=D + 1)[:, :, :D],
                        in_=ps[:, :GC].rearrange("p (h d) -> p h d", d=D))

                # attention for query chunk ic=tcix
                ic = tcix
                iq = slice(512 * ic, 512 * (ic + 1))
                njb = 4 * ic + 4                     # causal: key tiles 0..4ic+3
                for hp in range(2):                  # head pair = partition block
                    ys = [yav.tile([P, 512], fp32, tag="yav", name=f"yav_{hp}_{ic}_{u2}") for u2 in range(2)]
                    for s in range(njb // 2):        # spans of 2 key tiles
                        jbs = (2 * s, 2 * s + 1)
                        # hi diagonal span (mask tiles m=2,3): only queries
                        # >= 256+j / 384+j survive, so compute the partial
                        # column ranges only (QK, exp, mask, AV all shrunk)
                        hi_diag = (jbs[0] - 4 * ic == 2)
                        lo_diag = (jbs[0] - 4 * ic == 0)
                        # per k: (query-offset, width) within this 512-chunk
                        qr_ = ((256, 256), (384, 128)) if hi_diag else ((0, 512), (0, 512))
                        # AV/mask ranges can be tighter than QK/exp (exp output
                        # outside them is never read)
                        av_ = ((0, 512), (128, 384)) if lo_diag else qr_
                        spans, pts = [], []
                        # both heads' QK interleaved: the pair's K=64 matmuls
                        # sit in PE row groups 0-1 / 2-3 (lhsT base partition)
                        # and overlap in the array
                        for u in range(2):
                            spans.append(sspan.tile([P, 1024], fp32, tag="sspan",
                                                    name=f"span_{hp}_{ic}_{s}_{u}"))
                        for k, jb in enumerate(jbs):
                            qo, qw = qr_[k]
                            for u in range(2):
                                hb = 64 * u
                                nc.tensor.matmul(
                                    spans[u][:, 512 * k + qo:512 * k + qo + qw],
                                    qk_rope[2 + hp][hb:hb + 64, 128 * jb:128 * (jb + 1)],
                                    qk_rope[hp][hb:hb + 64, 512 * ic + qo:512 * ic + qo + qw],
                                    start=True, stop=True)
                        for u in range(2):
                            pt = ptpool.tile([P, 1024], bf16, tag="pt",
                                             name=f"pt_{hp}_{ic}_{s}_{u}")
                            if hi_diag:
                                for k in range(2):
                                    qo, qw = qr_[k]
                                    nc.scalar.activation(
                                        pt[:, 512 * k + qo:512 * k + qo + qw],
                                        spans[u][:, 512 * k + qo:512 * k + qo + qw],
                                        EXP, scale=0.125)
                            else:
                                nc.scalar.activation(pt[:], spans[u][:], EXP, scale=0.125)
                            pts.append(pt)
                        for u in range(2):
                            for k, jb in enumerate(jbs):
                                m = jb - 4 * ic
                                if m >= 0:           # diagonal tile -> causal mask
                                    qo, qw = av_[k]
                                    nc.vector.tensor_tensor(
                                        pts[u][:, 512 * k + qo:512 * k + qo + qw],
                                        pts[u][:, 512 * k + qo:512 * k + qo + qw],
                                        mask_sb[:, m, qo:qo + qw], MUL)
                        for u in range(2):
                            h = 2 * hp + u
                            for k, jb in enumerate(jbs):
                                qo, qw = av_[k]
                                nc.tensor.matmul(
                                    ys[u][:D + 1, qo:qo + qw],
                                    v_aug[:, jb, (D + 1) * h:(D + 1) * (h + 1)],
                                    pts[u][:, 512 * k + qo:512 * k + qo + qw],
                                    start=(jb == 0), stop=(jb == njb - 1))
                    for u in range(2):
                        recip = work.tile([1, 512], bf16, tag="recip", name=f"recip_{hp}_{ic}_{u}")
                        with nc.allow_low_precision(reason="1/l in bf16: 0.4% on softmax scale is within tolerance"):
                            nc.vector.reciprocal(recip[:], ys[u][D:D + 1, :])
                        rb = sspan.tile([P, 512], fp32, tag="sspan", name=f"rb_{hp}_{ic}_{u}")
                        nc.tensor.matmul(rb[:D, :], ones_sb[:], recip[:], start=True, stop=True)
                        rbs = work.tile([D, 512], fp32, tag="rbs", name=f"rbs_{hp}_{ic}_{u}")
                        nc.any.tensor_copy(out=rbs[:], in_=rb[:D, :])
                        nc.vector.tensor_tensor(
                            ynorm[hp][64 * u:64 * u + D, iq], ys[u][:D, :], rbs[:], MUL)
            # output projection emitted last (= lowest scheduler priority):
            # its matmuls fill PE gaps in the ACT-paced attention stretches,
            # on the qkv PSUM slots that are idle by then
            for pic in range(NIC):
                emit_proj(pic, qkvp, "qkvp")


    _split_excess_waits(nc)
    return nc


def _split_excess_waits(nc, maxw=1):
    """Walrus codegen rejects instructions carrying >1 sem wait; move excess
    waits onto no-ops inserted immediately before, on the same engine."""
    import concourse.mybir as mybir
    n = 0
    for f in nc.m.functions:
        for bb in f.blocks:
            new = []
            for inst in bb.instructions:
                si = getattr(inst, "sync_info", None)
                if si is not None and si.on_wait and len(si.on_wait) > maxw:
                    waits = list(si.on_wait)
                    excess, keep = waits[:-maxw], waits[-maxw:]
                    for i in range(0, len(excess), maxw):
                        new.append(mybir.InstNoOp(
                            name=f"{inst.name}_wsp{n}_{i}", engine=inst.engine,
                            bass_nofuse=True,
                            sync_info=mybir.SyncInfo(on_wait=excess[i:i + maxw],
                                                     on_update=[])))
                    si.on_wait = keep
                    n += 1
                new.append(inst)
            bb.instructions[:] = new
    return n


def _get_runner():
    """Build the Bass program once and wrap it in a shard_map-jitted callable
    over the 8 cores (mirrors concourse.bass2jax.run_bass_via_pjrt)."""
    if "runner" in _CACHE:
        return _CACHE["runner"]
    import jax
    import numpy as _np
    from jax.sharding import Mesh, PartitionSpec
    from jax.experimental.shard_map import shard_map
    import concourse.mybir as mybir
    from concourse.bass2jax import _bass_exec_p, install_neuronx_cc_hook

    install_neuronx_cc_hook()
    from concourse.bass2jax import partition_id_tensor
    nc = _build_program()

    part_name = nc.partition_id_tensor.name if nc.partition_id_tensor else None
    in_names, out_names, out_avals = [], [], []
    for alloc in nc.m.functions[0].allocations:
        if not isinstance(alloc, mybir.MemoryLocationSet):
            continue
        name = alloc.memorylocations[0].name
        if alloc.kind == "ExternalInput":
            if name != part_name:
                in_names.append(name)
        elif alloc.kind == "ExternalOutput":
            out_names.append(name)
            out_avals.append(jax.core.ShapedArray(
                tuple(alloc.tensor_shape), mybir.dt.np(alloc.dtype)))
    n_params = len(in_names)
    all_names = in_names + out_names
    if part_name is not None:
        all_names = all_names + [part_name]

    def _body(*args):
        operands = list(args)
        if part_name is not None:
            operands.append(partition_id_tensor())
        outs = _bass_exec_p.bind(
            *operands, out_avals=tuple(out_avals), in_names=tuple(all_names),
            out_names=tuple(out_names), lowering_input_output_aliases=(),
            sim_require_finite=True, sim_require_nnan=True, nc=nc)
        return tuple(outs)

    devices = jax.devices()[:N_CORES]
    mesh = Mesh(_np.asarray(devices), ("core",))
    n_outs = len(out_names)
    sharded = jax.jit(
        shard_map(_body, mesh=mesh,
                  in_specs=(PartitionSpec("core"),) * (n_params + n_outs),
                  out_specs=(PartitionSpec("core"),) * n_outs,
                  check_rep=False),
        donate_argnums=tuple(range(n_params, n_params + n_outs)),
        keep_unused=True)

    runner = (sharded, in_names, out_names, out_avals)
    _CACHE["runner"] = runner
    return runner


def _prepare_core_inputs(x, w_qkv, w_proj):
    bf = ml_dtypes.bfloat16
    cosT, sinT = _CACHE.setdefault("rope", _rope_tables())
    # q_rope = q*cos + R(q * sinP) with sinP = half-swapped sin:
    #   (R(q*sinP))[d] = sign_d * q[s(d)] * sinP[s(d)] = rot_half(q)[d] * sin[d]
    sinP = np.concatenate([sinT[D // 2:D], sinT[:D // 2]], axis=0)
    sinP = np.concatenate([sinP, sinP], axis=0)[:P]
    cosT, sinT = cosT.astype(bf), sinP.astype(bf)
    # lhsT for the on-device rotate-half matmul: out = rmat.T @ q = R_pair @ q
    R = np.zeros((D, D), np.float32)
    for d in range(D // 2):
        R[d, d + D // 2] = -1.0
        R[d + D // 2, d] = 1.0
    R_pair = np.zeros((P, P), np.float32)
    R_pair[:D, :D] = R
    R_pair[D:, D:] = R
    rmat = np.ascontiguousarray(R_pair.T).astype(bf)
    masks = _CACHE.setdefault("masks", _masks()).astype(bf)
    xTs = [np.ascontiguousarray(x[b].T).astype(bf) for b in range(B)]
    per_core = []
    for core in range(N_CORES):
        b, g = divmod(core, 4)
        rows = slice(GC * g, GC * (g + 1))
        wq = w_qkv[0 * C:1 * C][rows]
        wk = w_qkv[1 * C:2 * C][rows]
        wv = w_qkv[2 * C:3 * C][rows]
        wT = np.ascontiguousarray(
            np.concatenate([wq, wk, wv], axis=0).T).astype(bf)      # [C, 768]
        wpT = np.ascontiguousarray(w_proj[:, rows].T).astype(bf)    # [256, C]
        per_core.append({
            "xT": xTs[b], "wT": wT, "wpT": wpT, "rmat": rmat,
            "cosT": cosT, "sinT": sinT, "masks": masks})
    return per_core


def _run_cores(per_core):
    from concourse import bass_utils
    if "nc" not in _CACHE:
        from concourse.bass2jax import install_neuronx_cc_hook
        install_neuronx_cc_hook()
        _CACHE["nc"] = _build_program()
    res = bass_utils.run_bass_kernel_spmd(
        _CACHE["nc"], per_core, core_ids=list(range(N_CORES)))
    return res.results


def kernel(x, w_qkv, w_proj):
    x = np.asarray(x, dtype=np.float32)
    w_qkv = np.asarray(w_qkv, dtype=np.float32)
    w_proj = np.asarray(w_proj, dtype=np.float32)
    per_core = _prepare_core_inputs(x, w_qkv, w_proj)
    results = _run_cores(per_core)
    out = np.zeros((B, T, C), dtype=np.float32)
    for core in range(N_CORES):
        b = core // 4
        out[b] += results[core]["out"]
    return out

